# revision 1
# baseline (speedup 1.0000x reference)
"""Trainium2 Bass kernel for nn_Attention (B=4, N=1024, DIM=1024, H=16).

Sharding: 8 cores = 4 batches x 2 query-halves of 512 rows each. No
collectives — each core recomputes its batch's K/V projections.

Matmuls run in bf16 (inputs pre-cast on host / in DVE copies) with fp32
PSUM accumulation.

Per-core pipeline:
  phase 1: KpT[d,k], Vp[k,(h,65)] (65th col = kmask01 -> softmax denom),
           QpT[d,q], Qp[q,d] projections.
  phase 2: per head pair: S^T[k,q] = Kh.Qh^T -> exp (no max subtraction;
           scores are tiny) -> (A.V | denom) via 65-wide lhsT -> PE
           transpose back to [q, 64] -> divide by denom -> O[q,d].
  phase 3: residual + LN1 -> transpose -> fc_o -> exact GELU + residual
           -> LN2 -> * qmask01 -> out.

Masking: masked-K rows are zeroed in Vp and in the denom column (exactly
the reference's post-softmax zeroing); masked-Q rows flow through as
finite garbage and are zeroed by the final qmask multiply.

Inputs are packed host-side so each phase's SBUF loads are a single DMA
(one completion semaphore -> at most one extra wait per matmul).
"""

import numpy as np
import ml_dtypes
from contextlib import ExitStack

import concourse.bass as bass
import concourse.bacc as bacc
import concourse.mybir as mybir
import concourse.tile as tile
from concourse.bass_utils import run_bass_kernel_spmd
from concourse.masks import make_identity

FP = mybir.dt.float32
BF = mybir.dt.bfloat16
AF = mybir.ActivationFunctionType
ALU = mybir.AluOpType

DIM = 1024
H = 16
DH = 64
B = 4
N = 1024          # keys per batch
NQ = 512          # queries per core
P = 128
NDT = DIM // P    # 8 feature tiles
NKT = N // P      # 8 key tiles
NQT = NQ // P     # 4 query tiles
EPS = 1e-5

_CACHED_NC = None


def _ln_apply(nc, pool, x_ap, out_ap, eps_sb, extra_scale=None):
    """LayerNorm (g=1, b=0) of x_ap [128, 1024] into out_ap."""
    stats = pool.tile([P, 2, 6], FP, tag="ln_stats", name="ln_stats", bufs=4)
    mv = pool.tile([P, 2], FP, tag="ln_mv", name="ln_mv", bufs=4)
    xg = x_ap.rearrange("p (s d) -> p s d", s=2)
    for s in range(2):
        nc.vector.bn_stats(out=stats[:, s, :], in_=xg[:, s, :])
    nc.vector.bn_aggr(out=mv, in_=stats)
    sd = pool.tile([P, 1], FP, tag="ln_sd", name="ln_sd", bufs=4)
    nc.scalar.activation(out=sd, in_=mv[:, 1:2], func=AF.Sqrt, bias=eps_sb)
    rstd = pool.tile([P, 1], FP, tag="ln_rstd", name="ln_rstd", bufs=4)
    nc.vector.reciprocal(out=rstd, in_=sd)
    if extra_scale is not None:
        nc.vector.tensor_mul(rstd, rstd, extra_scale)
    nc.vector.tensor_scalar(
        out=out_ap, in0=x_ap, scalar1=mv[:, 0:1], scalar2=rstd,
        op0=ALU.subtract, op1=ALU.mult,
    )


def build_nc(phases=3):
    nc = bacc.Bacc(None, target_bir_lowering=False, debug=True)
    # packa: [P, 16, N] bf16 — j 0..7 = K.T row-tiles, 8..15 = (Wk.T/32) row-tiles
    packa = nc.declare_dram_parameter("packa", [P, 2 * NDT, N], BF, isOutput=False)
    packb = nc.declare_dram_parameter("packb", [P, 2 * NDT, N], BF, isOutput=False)
    # packc: [P, 8, 1536] — [:, j, 0:512] = Q.T row-tiles, [:, j, 512:1536] = Wq.T
    packc = nc.declare_dram_parameter("packc", [P, NDT, NQ + DIM], BF, isOutput=False)
    wo = nc.declare_dram_parameter("wo", [P, NDT, DIM], BF, isOutput=False)
    # maskd: [P, 12] f32 — cols 0..7 = kmask01 tiles, 8..11 = qmask01 tiles
    maskd = nc.declare_dram_parameter("maskd", [P, NKT + NQT], FP, isOutput=False)
    out = nc.declare_dram_parameter("out", [NQ, DIM], FP, isOutput=True)

    with ExitStack() as ctx:
        tc = ctx.enter_context(tile.TileContext(nc))
        persist = ctx.enter_context(tc.tile_pool(name="persist", bufs=1))

        KpT = [persist.tile([P, N], BF, tag=f"kpt{i}", name=f"kpt{i}") for i in range(NDT)]
        Vp = [persist.tile([P, H, DH + 1], BF, tag=f"vp{i}", name=f"vp{i}") for i in range(NKT)]
        Qp = [persist.tile([P, DIM], BF, tag=f"qp{t}", name=f"qp{t}") for t in range(NQT)]
        Ob = persist.tile([P, NQT, DIM], FP, tag="ob", name="ob")
        identb = persist.tile([P, P], BF, tag="identb", name="identb")
        make_identity(nc, identb)
        eps_sb = persist.tile([P, 1], FP, tag="eps", name="eps_sb")
        nc.vector.memset(eps_sb, EPS)
        mask_sb = persist.tile([P, NKT + NQT], FP, tag="maskd", name="mask_sb")
        pa = persist.tile([P, 2 * NDT, N], BF, tag="pa", name="pa_sb")
        pb = persist.tile([P, 2 * NDT, N], BF, tag="pb", name="pb_sb")
        pc = persist.tile([P, NDT, NQ + DIM], BF, tag="pc", name="pc_sb")
        wo_sb3 = persist.tile([P, NDT, DIM], BF, tag="wosb", name="wosb3")
        nc.sync.dma_start(out=mask_sb, in_=maskd[:, :])
        km_sb = mask_sb[:, 0:NKT]
        qm_sb = mask_sb[:, NKT:NKT + NQT]

        # ---------- phase 1a: KpT[dout, k] ----------
        with tc.tile_pool(name="p1ap", bufs=4, space="PSUM") as p1ap:
            pa_d = packa[:, :, :].rearrange("p (x j) n -> p j x n", x=2)
            pa_v = pa.rearrange("p (x j) n -> p j x n", x=2)
            for j in range(NDT):
                nc.sync.dma_start(out=pa_v[:, j], in_=pa_d[:, j])
            for i in range(NDT):
                for c in range(2):
                    ps = p1ap.tile([P, 512], FP, tag="ps", name="ps1a")
                    for j in range(NDT):
                        nc.tensor.matmul(ps, pa[:, NDT + j, i * P:(i + 1) * P],
                                         pa[:, j, c * 512:(c + 1) * 512],
                                         start=(j == 0), stop=(j == NDT - 1))
                    nc.vector.tensor_copy(KpT[i][:, c * 512:(c + 1) * 512], ps)

        # ---------- phase 1c: QpT[dout, q] and Qp[q, dout] ----------
        midctx = ExitStack()
        midpool = midctx.enter_context(tc.tile_pool(name="mid", bufs=1))
        QpT = [midpool.tile([P, NQ], BF, tag=f"qpt{i}", name=f"qpt{i}") for i in range(NDT)]
        with tc.tile_pool(name="p1cp", bufs=2, space="PSUM") as p1cp:
            for j in range(NDT):
                nc.sync.dma_start(out=pc[:, j], in_=packc[:, j, :])
            qt_sb = [pc[:, j, 0:NQ] for j in range(NDT)]
            wq_sb = [pc[:, j, NQ:NQ + DIM] for j in range(NDT)]
            for i in range(NDT):
                ps = p1cp.tile([P, 512], FP, tag="ps", name="ps1c")
                for j in range(NDT):
                    nc.tensor.matmul(ps, wq_sb[j][:, i * P:(i + 1) * P], qt_sb[j],
                                     start=(j == 0), stop=(j == NDT - 1))
                nc.vector.tensor_copy(QpT[i], ps)
            for t in range(NQT):
                for i in range(NDT):
                    tq = p1cp.tile([P, P], BF, tag="tq", name=f"tq_{t}_{i}")
                    nc.tensor.transpose(tq, QpT[i][:, t * P:(t + 1) * P], identb)
                    nc.vector.tensor_copy(Qp[t][:, i * P:(i + 1) * P], tq)
            # head pair 0: scores+exp early so ACT overlaps phase 1b
            with tc.tile_pool(name="spre", bufs=2, space="PSUM") as spre:
                es_pre = []
                for j in range(NKT):
                    sp = spre.tile([P, 2, NQ], FP, tag="spp", name=f"spp{j}")
                    for s in range(2):
                        po = DH * s
                        nc.tensor.matmul(
                            sp[:, s, :],
                            KpT[0][po:po + DH, j * P:(j + 1) * P],
                            QpT[0][po:po + DH, :],
                            start=True, stop=True)
                    es = midpool.tile([P, 2, NQ], BF, tag=f"esp{j}", name=f"esp{j}")
                    nc.scalar.activation(out=es, in_=sp, func=AF.Exp)
                    es_pre.append(es)

        if phases < 2:
            midctx.close()
            return _finish(nc)
        # ---------- phase 2: attention, head pairs ----------
        with tc.tile_pool(name="p2es", bufs=1) as p2es, \
             tc.tile_pool(name="p2sb", bufs=2) as p2sb, \
             tc.tile_pool(name="p2sm", bufs=8) as p2sm, \
             tc.tile_pool(name="sps", bufs=2, space="PSUM") as sps:
            # head pair 1: scores+exp early too (tiles from p2es pool)
            es_pre1 = []
            for j in range(NKT):
                sp = sps.tile([P, 2, NQ], FP, tag="sp", name=f"sp1_{j}")
                for s in range(2):
                    po = DH * s
                    nc.tensor.matmul(
                        sp[:, s, :],
                        KpT[1][po:po + DH, j * P:(j + 1) * P],
                        QpT[1][po:po + DH, :],
                        start=True, stop=True)
                es = p2es.tile([P, 2, NQ], BF, tag=f"es{j}", name=f"es1_{j}")
                nc.scalar.activation(out=es, in_=sp, func=AF.Exp)
                es_pre1.append(es)
            # ---------- phase 1b: Vp[k, dout], masked, 65-col head layout ----------
            with tc.tile_pool(name="p1bp", bufs=4, space="PSUM") as p1bp:
                pb_d = packb[:, :, :].rearrange("p (x j) n -> p j x n", x=2)
                pb_v = pb.rearrange("p (x j) n -> p j x n", x=2)
                for j in range(NDT):
                    nc.sync.dma_start(out=pb_v[:, j], in_=pb_d[:, j])
                for c in range(2):
                    for i in range(NKT):
                        ps = p1bp.tile([P, 512], FP, tag="ps", name="ps1b")
                        for j in range(NDT):
                            nc.tensor.matmul(ps, pb[:, j, i * P:(i + 1) * P],
                                             pb[:, NDT + j, c * 512:(c + 1) * 512],
                                             start=(j == 0), stop=(j == NDT - 1))
                        nc.vector.tensor_scalar_mul(
                            out=Vp[i][:, 8 * c:8 * c + 8, 0:DH],
                            in0=ps.rearrange("p (h d) -> p h d", h=8),
                            scalar1=km_sb[:, i:i + 1])
                for i in range(NKT):
                    nc.vector.tensor_copy(Vp[i][:, :, DH:DH + 1],
                                          km_sb[:, i:i + 1].to_broadcast((P, H, 1)))


            avtp = ExitStack()
            avs = avtp.enter_context(tc.tile_pool(name="avs", bufs=3, space="PSUM"))
            tps = avtp.enter_context(tc.tile_pool(name="tps", bufs=1, space="PSUM"))
            for hp in range(H // 2):
                avps = [avs.tile([DH + 1, NQ], FP, tag="av", name=f"av{hp}_{s}")
                        for s in range(2)]
                for j in range(NKT):
                    if hp == 0:
                        es = es_pre[j]
                    elif hp == 1:
                        es = es_pre1[j]
                    else:
                        sp = sps.tile([P, 2, NQ], FP, tag="sp", name=f"sp{hp}_{j}")
                        for s in range(2):
                            po = DH * s
                            nc.tensor.matmul(
                                sp[:, s, :],
                                KpT[hp][po:po + DH, j * P:(j + 1) * P],
                                QpT[hp][po:po + DH, :],
                                start=True, stop=True)
                        es = p2es.tile([P, 2, NQ], BF, tag=f"es{j}", name=f"es{hp}_{j}")
                        nc.scalar.activation(out=es, in_=sp, func=AF.Exp)
                    for s in range(2):
                        h = 2 * hp + s
                        nc.tensor.matmul(avps[s], Vp[j][:, h, :], es[:, s, :],
                                         start=(j == 0), stop=(j == NKT - 1))
                for s in range(2):
                    h = 2 * hp + s
                    avsb = p2sb.tile([DH + 1, NQ], BF, tag="avsb", name=f"avsb{hp}_{s}")
                    nc.vector.tensor_copy(avsb, avps[s])
                    tpg = tps.tile([P, NQT, DH + 2], BF, tag="tp", name=f"tp{hp}_{s}")
                    for t in range(NQT):
                        nc.tensor.matmul(tpg[:, t, 0:DH + 1], avsb[:, t * P:(t + 1) * P],
                                         identb[0:DH + 1, 0:DH + 1],
                                         is_transpose=True,
                                         start=(t == 0), stop=(t == NQT - 1))
                    osb = p2sm.tile([P, NQT, DH + 2], BF, tag="osb", name=f"osb{hp}_{s}")
                    nc.vector.tensor_copy(osb[:, :, 0:DH + 1], tpg[:, :, 0:DH + 1])
                    dr = p2sm.tile([P, NQT, 1], FP, tag="dr", name=f"dr{hp}_{s}")
                    nc.vector.reciprocal(out=dr, in_=osb[:, :, DH:DH + 1])
                    nc.vector.tensor_mul(
                        Ob[:, :, h * DH:(h + 1) * DH],
                        osb[:, :, 0:DH],
                        dr.to_broadcast((P, NQT, DH)))
            avtp.close()
        midctx.close()
        if phases < 3:
            return _finish(nc)

        # ---------- phase 3: residual + LN1 + fc_o + GELU + LN2 ----------
        with tc.tile_pool(name="p3", bufs=1) as p3, \
             tc.tile_pool(name="p3s", bufs=1) as p3s, \
             tc.tile_pool(name="p3p", bufs=4, space="PSUM") as p3p, \
             tc.tile_pool(name="tps3", bufs=4, space="PSUM") as tps3:
            nc.sync.dma_start(out=wo_sb3, in_=wo[:, :, :])
            wo_sb = [wo_sb3[:, j] for j in range(NDT)]
            O1 = [p3.tile([P, DIM], BF, tag=f"o1_{t}", name=f"o1_{t}") for t in range(NQT)]
            OTb = p3.tile([P, NDT, NQ], BF, tag="otb", name="otb")
            OT = [OTb[:, i] for i in range(NDT)]
            for t in range(NQT):
                r1 = p3s.tile([P, DIM], FP, tag="r1", name=f"r1_{t}", bufs=3)
                nc.vector.tensor_add(r1, Qp[t], Ob[:, t])
                _ln_apply(nc, p3s, r1, O1[t], eps_sb)
                tp = tps3.tile([P, NDT, P], BF, tag="tp3", name=f"tp3_{t}")
                for i in range(NDT):
                    nc.tensor.matmul(tp[:, i, :], O1[t][:, i * P:(i + 1) * P], identb,
                                     is_transpose=True,
                                     start=(i == 0), stop=(i == NDT - 1))
                nc.vector.tensor_copy(OTb[:, :, t * P:(t + 1) * P], tp)
            for t in range(NQT):
                g = p3s.tile([P, DIM], FP, tag="g", name=f"g_{t}", bufs=2)
                r2 = p3s.tile([P, DIM], FP, tag="r1", name=f"r2_{t}", bufs=3)
                for c in range(2):
                    ps = p3p.tile([P, 512], FP, tag="hps", name=f"hps_{t}_{c}")
                    for i in range(NDT):
                        nc.tensor.matmul(ps, OT[i][:, t * P:(t + 1) * P],
                                         wo_sb[i][:, c * 512:(c + 1) * 512],
                                         start=(i == 0), stop=(i == NDT - 1))
                    nc.scalar.activation(out=g[:, c * 512:(c + 1) * 512], in_=ps, func=AF.Gelu)
                    nc.vector.tensor_add(r2[:, c * 512:(c + 1) * 512], O1[t][:, c * 512:(c + 1) * 512],
                                         g[:, c * 512:(c + 1) * 512])
                fin = p3s.tile([P, DIM], FP, tag="g", name=f"fin_{t}", bufs=2)
                _ln_apply(nc, p3s, r2, fin, eps_sb, extra_scale=qm_sb[:, t:t + 1])
                nc.sync.dma_start(out=out[t * P:(t + 1) * P, :], in_=fin)

    return _finish(nc)


def _finish(nc):
    nc.compile()
    return nc


def _get_nc():
    global _CACHED_NC
    if _CACHED_NC is None:
        _CACHED_NC = build_nc()
    return _CACHED_NC


def _pack_rows(mats):
    """[t*128, n] row-major mats -> one [128, sum_t, n] array (j-tile minor)."""
    blocks = []
    for m in mats:
        r, n = m.shape
        blocks.append(m.reshape(r // P, P, n).transpose(1, 0, 2))
    return np.concatenate(blocks, axis=1)


def _make_in_maps(inputs):
    Q, K, V = inputs["Q"], inputs["K"], inputs["V"]
    mask_Q, mask_K = inputs["mask_Q"], inputs["mask_K"]
    bf = ml_dtypes.bfloat16
    sc = 1.0 / np.sqrt(np.float32(DIM))
    wqT = np.ascontiguousarray(inputs["Wq"].T)
    wkT = np.ascontiguousarray(inputs["Wk"].T) * sc
    wvT = np.ascontiguousarray(inputs["Wv"].T)
    woT = np.ascontiguousarray(_pack_rows([np.ascontiguousarray(inputs["Wo"].T)])).astype(bf)
    in_maps = []
    for c in range(8):
        b, q0 = c // 2, (c % 2) * NQ
        kt = np.ascontiguousarray(K[b].T)
        vt = np.ascontiguousarray(V[b].T)
        qt = np.ascontiguousarray(Q[b, q0:q0 + NQ, :].T)
        packa = np.ascontiguousarray(_pack_rows([kt, wkT])).astype(bf)
        packb = np.ascontiguousarray(_pack_rows([vt, wvT])).astype(bf)
        qt_j = qt.reshape(NDT, P, NQ).transpose(1, 0, 2)
        wq_j = wqT.reshape(NDT, P, DIM).transpose(1, 0, 2)
        packc = np.ascontiguousarray(np.concatenate([qt_j, wq_j], axis=2)).astype(bf)
        km01 = np.where(mask_K[b], 0.0, 1.0).astype(np.float32)
        qm01 = np.where(mask_Q[b, q0:q0 + NQ], 0.0, 1.0).astype(np.float32)
        maskd = np.concatenate([km01.reshape(NKT, P).T,
                                qm01.reshape(NQT, P).T], axis=1)
        in_maps.append({
            "packa": packa, "packb": packb, "packc": packc, "wo": woT,
            "maskd": np.ascontiguousarray(maskd),
        })
    return in_maps


def _assemble(results):
    out = np.empty((B, 1024, DIM), np.float32)
    for c in range(8):
        b, q0 = c // 2, (c % 2) * NQ
        out[b, q0:q0 + NQ, :] = results[c]["out"]
    return out


def kernel(**inputs):
    nc = _get_nc()
    res = run_bass_kernel_spmd(nc, _make_in_maps(inputs), core_ids=list(range(8)))
    return _assemble(res.results)


def kernel_profiled(inputs, **kw):
    nc = _get_nc()
    res = run_bass_kernel_spmd(nc, _make_in_maps(inputs),
                               core_ids=list(range(8)), trace=True, **kw)
    return _assemble(res.results), res



# revision 7
# speedup vs baseline: 1.2730x; 1.2730x over previous
"""Trainium2 Bass kernel for nn_Attention (B=4, N=1024, DIM=1024, H=16).

Sharding: 8 cores = 4 batches x 2 query-halves of 512 rows each. No
collectives - each core recomputes its batch's K/V projections.

Key design (cost model: matmul cost = moving rows x 0.4167ns x cyc/row;
fp8e4m3 DoubleRow = 0.5 cyc/row with 2x128 contraction = 4x bf16):
  - K/V projections and a scores-only Q projection run in fp8 DoubleRow.
    Weights are pre-scaled x16 (Wq/Wk) so fp8 quantization stays in the
    normal range; the 1/(32*16*16) total score scale folds into the exp.
  - Scores use a DMA-shuffled [32, 2, *] layout (dims interleaved into
    partition+slot) so the 64-dim head contraction runs as one DoubleRow
    matmul; A*V uses slot = key-tile pairs with fp8 es written directly
    by the exp.
  - The residual path stays bf16: precise Qp projection (direct [q, d]
    layout, no transposes) and bf16 fc_o. Attention output is ~30x
    smaller than Qp, so fp8 noise there is harmless.
  - LayerNorm rstd = exp(-0.5*ln(var+eps)) keeps ACT on the exp/ln
    table set (3 table loads total: exp/ln -> gelu -> exp/ln).
  - Masking: masked K rows are zeroed via the km scalar in the V
    evacuation and excluded from the softmax denominator (65th V
    column = km01); masked Q rows are zeroed by the final qmask scale.
"""

import numpy as np
import ml_dtypes
from contextlib import ExitStack

import concourse.bass as bass
import concourse.bacc as bacc
import concourse.mybir as mybir
import concourse.tile as tile
from concourse.bass_utils import run_bass_kernel_spmd
from concourse.masks import make_identity

FP = mybir.dt.float32
BF = mybir.dt.bfloat16
F8 = mybir.dt.float8e4
AF = mybir.ActivationFunctionType
ALU = mybir.AluOpType
DRM = mybir.MatmulPerfMode.DoubleRow

DIM = 1024
H = 16
DH = 64
B = 4
N = 1024          # keys per batch
NQ = 512          # queries per core
P = 128
NDT = DIM // P    # 8 feature tiles
NKT = N // P      # 8 key tiles
NQT = NQ // P     # 4 query tiles
NG = 4            # DoubleRow contraction groups (4 x 2 x 128 = 1024)
EPS = 1e-5
ESCALE = 1.0 / 8192.0   # 1/(sqrt(DIM) * 16 * 16)

_CACHED_NC = None


def build_nc():
    nc = bacc.Bacc(None, target_bir_lowering=False, debug=True)
    qt8 = nc.declare_dram_parameter("qt8", [P, NG, 2, NQ], F8, isOutput=False)
    wq8 = nc.declare_dram_parameter("wq8", [P, NG, 2, DIM], F8, isOutput=False)
    kt8 = nc.declare_dram_parameter("kt8", [P, NG, 2, N], F8, isOutput=False)
    wk8 = nc.declare_dram_parameter("wk8", [P, NG, 2, DIM], F8, isOutput=False)
    wv8 = nc.declare_dram_parameter("wv8", [P, NG, 2, DIM], F8, isOutput=False)
    vt8 = nc.declare_dram_parameter("vt8", [P, NG, 2, N], F8, isOutput=False)
    qtb = nc.declare_dram_parameter("qtb", [P, NDT, NQ], BF, isOutput=False)
    wqb = nc.declare_dram_parameter("wqb", [P, NDT, DIM], BF, isOutput=False)
    wo = nc.declare_dram_parameter("wo", [P, NDT, DIM], BF, isOutput=False)
    # maskd: cols 0..7 = km01 key-tile columns, 8..11 = qm01 query-tile cols
    maskd = nc.declare_dram_parameter("maskd", [P, NKT + NQT], FP, isOutput=False)
    out = nc.declare_dram_parameter("out", [NQ, DIM], BF, isOutput=True)

    with ExitStack() as ctx:
        tc = ctx.enter_context(tile.TileContext(nc))
        persist = ctx.enter_context(tc.tile_pool(name="persist", bufs=1))

        # ---- persistent SBUF tiles ----
        mask_sb = persist.tile([P, NKT + NQT], FP, tag="mask", name="mask_sb")
        qt8_sb = persist.tile([P, NG, 2, NQ], F8, tag="qt8", name="qt8_sb")
        wq8_sb = persist.tile([P, NG, 2, DIM], F8, tag="wq8", name="wq8_sb")
        kt8_sb = persist.tile([P, NG, 2, N], F8, tag="kt8", name="kt8_sb")
        wk8_sb = persist.tile([P, NG, 2, DIM], F8, tag="wk8", name="wk8_sb")
        wv8_sb = persist.tile([P, NG, 2, DIM], F8, tag="wv8", name="wv8_sb")
        vt8_sb = persist.tile([P, NG, 2, N], F8, tag="vt8", name="vt8_sb")
        qtb_sb = persist.tile([P, NDT, NQ], BF, tag="qtb", name="qtb_sb")
        wqb_sb = persist.tile([P, NDT, DIM], BF, tag="wqb", name="wqb_sb")
        wo_sb = persist.tile([P, NDT, DIM], BF, tag="wo", name="wo_sb")
        Q8pT = [persist.tile([P, NQ], F8, tag=f"q8pt{i}", name=f"q8pt{i}")
                for i in range(NDT)]
        KpT8 = [persist.tile([P, N], F8, tag=f"kpt8{i}", name=f"kpt8{i}")
                for i in range(NDT)]
        QDR = [persist.tile([64, 2, NQ], F8, tag=f"qdr{i}", name=f"qdr{i}")
               for i in range(NDT)]
        KDR = [persist.tile([64, 2, N], F8, tag=f"kdr{i}", name=f"kdr{i}")
               for i in range(NDT)]
        # VDR[jp]: slot t = key-tile 2jp+t; per head 66 cols (64 dims,
        # col 64 = km01 denominator column, col 65 pad)
        VDR = [persist.tile([P, 2, H, DH + 2], F8, tag=f"vdr{j}", name=f"vdr{j}")
               for j in range(NKT // 2)]
        Qp = [persist.tile([P, DIM], BF, tag=f"qp{t}", name=f"qp{t}")
              for t in range(NQT)]
        Ob = persist.tile([P, NQT, DIM], BF, tag="ob", name="ob")
        O1 = [persist.tile([P, DIM], BF, tag=f"o1_{t}", name=f"o1_{t}")
              for t in range(NQT)]
        OTb = persist.tile([P, NDT, NQ], BF, tag="otb", name="otb")
        identb = persist.tile([P, P], BF, tag="identb", name="identb")
        make_identity(nc, identb)
        eps_sb = persist.tile([P, 1], FP, tag="eps", name="eps_sb")
        nc.vector.memset(eps_sb, EPS)

        km = mask_sb[:, 0:NKT]
        qm = mask_sb[:, NKT:NKT + NQT]

        # ---- input DMAs (SP, issue order = priority order) ----
        nc.sync.dma_start(out=mask_sb, in_=maskd[:, :])
        nc.sync.dma_start(out=qt8_sb, in_=qt8[:, :, :, :])
        nc.sync.dma_start(out=wq8_sb, in_=wq8[:, :, :, :])
        nc.sync.dma_start(out=kt8_sb, in_=kt8[:, :, :, :])
        nc.sync.dma_start(out=wk8_sb[:, :, :, 0:512], in_=wk8[:, :, :, 0:512])
        nc.sync.dma_start(out=wk8_sb[:, :, :, 512:1024], in_=wk8[:, :, :, 512:1024])
        nc.sync.dma_start(out=wv8_sb, in_=wv8[:, :, :, :])
        nc.sync.dma_start(out=vt8_sb, in_=vt8[:, :, :, :])
        nc.sync.dma_start(out=qtb_sb, in_=qtb[:, :, :])
        nc.sync.dma_start(out=wqb_sb[:, 0:4], in_=wqb[:, 0:4, :])
        nc.sync.dma_start(out=wqb_sb[:, 4:8], in_=wqb[:, 4:8, :])
        nc.sync.dma_start(out=wo_sb[:, 0:4], in_=wo[:, 0:4, :])
        nc.sync.dma_start(out=wo_sb[:, 4:8], in_=wo[:, 4:8, :])

        # ---- phase 1a: scores-Q projection (fp8 DR) ----
        with tc.tile_pool(name="q8ps", bufs=2, space="PSUM") as q8ps:
            for i in range(NDT):
                ps = q8ps.tile([P, NQ], FP, tag="ps", name=f"q8ps{i}")
                for g in range(NG):
                    nc.tensor.matmul(ps, wq8_sb[:, g, :, P * i:P * (i + 1)],
                                     qt8_sb[:, g, :, :],
                                     start=(g == 0), stop=(g == NG - 1),
                                     perf_mode=DRM)
                nc.vector.tensor_copy(Q8pT[i], ps)

        # ---- phase 1b: K projection (fp8 DR) ----
        with tc.tile_pool(name="kps", bufs=2, space="PSUM") as kps:
            for i in range(NDT):
                ps = kps.tile([P, 2, 512], FP, tag="ps", name=f"kps{i}")
                for c in range(2):
                    for g in range(NG):
                        nc.tensor.matmul(ps[:, c, :],
                                         wk8_sb[:, g, :, P * i:P * (i + 1)],
                                         kt8_sb[:, g, :, 512 * c:512 * c + 512],
                                         start=(g == 0), stop=(g == NG - 1),
                                         perf_mode=DRM)
                nc.vector.tensor_copy(KpT8[i], ps.rearrange("p c n -> p (c n)"))

        # ---- DR-layout shuffles (partition p -> (p//2, slot p%2)) ----
        for i in range(NDT):
            nc.sync.dma_start(out=QDR[i], in_=Q8pT[i][:, :])
        for i in range(NDT):
            nc.sync.dma_start(out=KDR[i], in_=KpT8[i][:, :])

        # ---- phase 2: attention (+ V proj and Qp proj interleaved) ----
        es_tiles = {}
        LAG = 2

        with tc.tile_pool(name="spp", bufs=2, space="PSUM") as spp, \
             tc.tile_pool(name="avp", bufs=1, space="PSUM") as avp, \
             tc.tile_pool(name="tpp", bufs=1, space="PSUM") as tpp, \
             tc.tile_pool(name="esp", bufs=3) as esp, \
             tc.tile_pool(name="p2sb", bufs=2) as p2sb:

            vctx = ExitStack()
            vps = vctx.enter_context(tc.tile_pool(name="vps", bufs=1, space="PSUM"))
            qpctx = ExitStack()
            qpps = None

            def scores_head(h):
                i, g = h // 2, 32 * (h % 2)
                for jp in range(NKT // 2):
                    sp = spp.tile([P, 2, NQ], FP, tag="sp", name=f"sp{h}_{jp}")
                    for s in range(2):
                        k = 2 * jp + s
                        nc.tensor.matmul(sp[:, s, :],
                                         KDR[i][g:g + 32, :, P * k:P * (k + 1)],
                                         QDR[i][g:g + 32, :, :],
                                         start=True, stop=True, perf_mode=DRM)
                    es = esp.tile([P, 2, NQ], F8, tag=f"es{jp}", name=f"es{h}_{jp}")
                    nc.scalar.activation(out=es, in_=sp, func=AF.Exp, scale=ESCALE)
                    es_tiles[(h, jp)] = es

            def av_head(h):
                av = avp.tile([DH + 1, NQ], FP, tag="av", name=f"av{h}")
                for jp in range(NKT // 2):
                    nc.tensor.matmul(av, VDR[jp][:, :, h, 0:DH + 1],
                                     es_tiles.pop((h, jp)),
                                     start=(jp == 0), stop=(jp == NKT // 2 - 1),
                                     perf_mode=DRM)
                avsb = p2sb.tile([DH + 1, NQ], BF, tag="avsb", name=f"avsb{h}")
                nc.vector.tensor_copy(avsb, av)
                tpg = tpp.tile([P, NQT, DH + 2], BF, tag="tpg", name=f"tpg{h}")
                for t in range(NQT):
                    nc.tensor.matmul(tpg[:, t, 0:DH + 1],
                                     avsb[:, P * t:P * (t + 1)],
                                     identb[0:DH + 1, 0:DH + 1],
                                     is_transpose=True,
                                     start=(t == 0), stop=(t == NQT - 1))
                osb = p2sb.tile([P, NQT, DH + 2], BF, tag="osb", name=f"osb{h}")
                nc.vector.tensor_copy(osb[:, :, 0:DH + 1], tpg[:, :, 0:DH + 1])
                dr = p2sb.tile([P, NQT, 1], BF, tag="dr", name=f"dr{h}")
                with nc.allow_low_precision(
                        reason="denom ~512, bf16 recip err 0.4% on a term 30x "
                               "smaller than the residual"):
                    nc.vector.reciprocal(out=dr, in_=osb[:, :, DH:DH + 1])
                nc.vector.tensor_mul(Ob[:, :, DH * h:DH * (h + 1)],
                                     osb[:, :, 0:DH],
                                     dr.to_broadcast((P, NQT, DH)))

            def v_proj():
                for i in range(NKT):
                    for c in range(2):
                        ps = vps.tile([P, 512], FP, tag="ps", name=f"vps{i}_{c}")
                        for g in range(NG):
                            nc.tensor.matmul(ps, vt8_sb[:, g, :, P * i:P * (i + 1)],
                                             wv8_sb[:, g, :, 512 * c:512 * (c + 1)],
                                             start=(g == 0), stop=(g == NG - 1),
                                             perf_mode=DRM)
                        nc.vector.tensor_scalar_mul(
                            out=VDR[i // 2][:, i % 2, 8 * c:8 * c + 8, 0:DH],
                            in0=ps.rearrange("p (h d) -> p h d", h=8),
                            scalar1=km[:, i:i + 1])
                # denominator columns (km01, excluded keys contribute 0)
                for jp in range(NKT // 2):
                    nc.vector.tensor_copy(
                        VDR[jp][:, :, :, DH:DH + 1],
                        km[:, 2 * jp:2 * jp + 2].to_broadcast((P, 2, H, 1)))

            def qp_proj_part(t):
                for c in range(2):
                    ps = qpps.tile([P, 512], FP, tag="ps", name=f"qpp{t}_{c}")
                    for j in range(NDT):
                        nc.tensor.matmul(ps, qtb_sb[:, j, P * t:P * (t + 1)],
                                         wqb_sb[:, j, 512 * c:512 * (c + 1)],
                                         start=(j == 0), stop=(j == NDT - 1))
                    nc.vector.tensor_copy(Qp[t][:, 512 * c:512 * (c + 1)], ps)

            for h in range(H):
                scores_head(h)
                if h == 1:
                    v_proj()
                if h == 5:
                    vctx.close()
                    qpps = qpctx.enter_context(
                        tc.tile_pool(name="qpps", bufs=2, space="PSUM"))
                if h >= LAG:
                    av_head(h - LAG)
                if 6 <= h <= 9:
                    qp_proj_part(h - 6)
            for h in range(H - LAG, H):
                av_head(h)
            qpctx.close()

        # ---- phase 3: residual + LN1 + fc_o + GELU + LN2 ----
        with tc.tile_pool(name="p3", bufs=1) as p3, \
             tc.tile_pool(name="p3s", bufs=2) as p3s, \
             tc.tile_pool(name="tpp3", bufs=2, space="PSUM") as tpp3, \
             tc.tile_pool(name="fps", bufs=2, space="PSUM") as fps:

            def ln_rstd(mv, tag, extra_scale=None):
                """rstd = exp(-0.5*ln(var+eps)) [* extra_scale]"""
                lnv = p3s.tile([P, 1], FP, tag="lnv", name=f"lnv{tag}", bufs=4)
                nc.scalar.activation(out=lnv, in_=mv[:, 1:2], func=AF.Ln,
                                     bias=eps_sb[:, 0:1])
                rstd = p3s.tile([P, 1], FP, tag="rstd", name=f"rstd{tag}", bufs=4)
                nc.scalar.activation(out=rstd, in_=lnv, func=AF.Exp, scale=-0.5)
                if extra_scale is not None:
                    nc.vector.tensor_mul(rstd, rstd, extra_scale)
                return rstd

            def ln_stats(x_ap, tag):
                stats = p3s.tile([P, 2, 6], FP, tag="st", name=f"st{tag}", bufs=4)
                xg = x_ap.rearrange("p (s d) -> p s d", s=2)
                for s in range(2):
                    nc.vector.bn_stats(out=stats[:, s, :], in_=xg[:, s, :])
                mv = p3s.tile([P, 2], FP, tag="mv", name=f"mv{tag}", bufs=4)
                nc.vector.bn_aggr(out=mv, in_=stats)
                return mv

            r1s = []
            for t in range(NQT):
                r1 = p3s.tile([P, DIM], BF, tag="r1", name=f"r1_{t}", bufs=4)
                nc.vector.tensor_add(r1, Qp[t], Ob[:, t])
                r1s.append(r1)
                mv = ln_stats(r1, f"a{t}")
                rstd = ln_rstd(mv, f"a{t}")
                nc.vector.tensor_scalar(
                    out=O1[t], in0=r1, scalar1=mv[:, 0:1], scalar2=rstd,
                    op0=ALU.subtract, op1=ALU.mult)
                tp = tpp3.tile([P, NDT, P], BF, tag="tp3", name=f"tp3_{t}")
                for i in range(NDT):
                    nc.tensor.matmul(tp[:, i, :], O1[t][:, P * i:P * (i + 1)],
                                     identb, is_transpose=True,
                                     start=(i == 0), stop=(i == NDT - 1))
                nc.vector.tensor_copy(OTb[:, :, P * t:P * (t + 1)], tp)

            r2s = []
            for t in range(NQT):
                r2 = p3s.tile([P, DIM], BF, tag="r2", name=f"r2_{t}", bufs=4)
                for c in range(2):
                    ps = fps.tile([P, 512], FP, tag="fps", name=f"fps{t}_{c}")
                    for i in range(NDT):
                        nc.tensor.matmul(ps, OTb[:, i, P * t:P * (t + 1)],
                                         wo_sb[:, i, 512 * c:512 * (c + 1)],
                                         start=(i == 0), stop=(i == NDT - 1))
                    g = p3s.tile([P, 512], BF, tag="g", name=f"g{t}_{c}", bufs=4)
                    nc.scalar.activation(out=g, in_=ps, func=AF.Gelu)
                    nc.vector.tensor_add(r2[:, 512 * c:512 * (c + 1)],
                                         O1[t][:, 512 * c:512 * (c + 1)], g)
                r2s.append(r2)

            for t in range(NQT):
                mv2 = ln_stats(r2s[t], f"b{t}")
                rstd2 = ln_rstd(mv2, f"b{t}", extra_scale=qm[:, t:t + 1])
                fin = p3s.tile([P, DIM], BF, tag="fin", name=f"fin_{t}", bufs=4)
                nc.vector.tensor_scalar(
                    out=fin, in0=r2s[t], scalar1=mv2[:, 0:1], scalar2=rstd2,
                    op0=ALU.subtract, op1=ALU.mult)
                nc.sync.dma_start(out=out[P * t:P * (t + 1), :], in_=fin)

    nc.compile()
    return nc


def _get_nc():
    global _CACHED_NC
    if _CACHED_NC is None:
        _CACHED_NC = build_nc()
    return _CACHED_NC


E4NP = ml_dtypes.float8_e4m3
BFNP = ml_dtypes.bfloat16


def _g_pack(m):
    """[1024 (din), cols] -> [128, 4, 2, cols] DoubleRow group layout."""
    cols = m.shape[1]
    return np.ascontiguousarray(
        m.reshape(NG, 2, P, cols).transpose(2, 0, 1, 3))


def _j_pack(m):
    """[8*128 rows, cols] -> [128, 8, cols] row-tile layout."""
    cols = m.shape[1]
    return np.ascontiguousarray(
        m.reshape(NDT, P, cols).transpose(1, 0, 2))


def _make_in_maps(inputs):
    Q, K, V = inputs["Q"], inputs["K"], inputs["V"]
    mask_Q, mask_K = inputs["mask_Q"], inputs["mask_K"]
    wqT = np.ascontiguousarray(inputs["Wq"].T.astype(np.float32))
    wkT = np.ascontiguousarray(inputs["Wk"].T.astype(np.float32))
    wvT = np.ascontiguousarray(inputs["Wv"].T.astype(np.float32))
    woT = np.ascontiguousarray(inputs["Wo"].T.astype(np.float32))

    wq8 = _g_pack(wqT * 16.0).astype(E4NP)
    wk8 = _g_pack(wkT * 16.0).astype(E4NP)
    wv8 = _g_pack(wvT).astype(E4NP)
    wqb = _j_pack(wqT).astype(BFNP)
    wob = _j_pack(woT).astype(BFNP)

    in_maps = []
    for c in range(8):
        b, q0 = c // 2, (c % 2) * NQ
        kT = np.ascontiguousarray(K[b].T)
        vT = np.ascontiguousarray(V[b].T)
        qT = np.ascontiguousarray(Q[b, q0:q0 + NQ, :].T)
        km01 = np.where(mask_K[b], 0.0, 1.0).astype(np.float32)
        qm01 = np.where(mask_Q[b, q0:q0 + NQ], 0.0, 1.0).astype(np.float32)
        maskd = np.concatenate([km01.reshape(NKT, P).T,
                                qm01.reshape(NQT, P).T], axis=1)
        in_maps.append({
            "qt8": _g_pack(qT).astype(E4NP),
            "wq8": wq8,
            "kt8": _g_pack(kT).astype(E4NP),
            "wk8": wk8,
            "wv8": wv8,
            "vt8": _g_pack(vT).astype(E4NP),
            "qtb": _j_pack(qT).astype(BFNP),
            "wqb": wqb,
            "wo": wob,
            "maskd": np.ascontiguousarray(maskd),
        })
    return in_maps


def _assemble(results):
    out = np.empty((B, 1024, DIM), np.float32)
    for c in range(8):
        b, q0 = c // 2, (c % 2) * NQ
        out[b, q0:q0 + NQ, :] = results[c]["out"].astype(np.float32)
    return out


def kernel(**inputs):
    nc = _get_nc()
    res = run_bass_kernel_spmd(nc, _make_in_maps(inputs), core_ids=list(range(8)))
    return _assemble(res.results)


def kernel_profiled(inputs, **kw):
    nc = _get_nc()
    res = run_bass_kernel_spmd(nc, _make_in_maps(inputs),
                               core_ids=list(range(8)), trace=True, **kw)
    return _assemble(res.results), res


# revision 12
# speedup vs baseline: 1.4045x; 1.1033x over previous
"""Trainium2 Bass kernel for nn_Attention (B=4, N=1024, DIM=1024, H=16).

Sharding: 8 cores = 4 batches x 2 query-halves of 512 rows each. No
collectives - each core recomputes its batch's K/V projections.

Key design (cost model: matmul cost = moving rows x 0.4167ns x cyc/row;
fp8e4m3 DoubleRow = 0.5 cyc/row with 2x128 contraction = 4x bf16):
  - K/V projections and a scores-only Q projection run in fp8 DoubleRow.
    Weights are pre-scaled x16 (Wq/Wk) so fp8 quantization stays in the
    normal range; the 1/(32*16*16) total score scale folds into the exp.
  - Scores use a DMA-shuffled [32, 2, *] layout (dims interleaved into
    partition+slot) so the 64-dim head contraction runs as one DoubleRow
    matmul; A*V uses slot = key-tile pairs with fp8 es written directly
    by the exp.
  - The residual path stays bf16: precise Qp projection (direct [q, d]
    layout, no transposes) and bf16 fc_o. Attention output is ~30x
    smaller than Qp, so fp8 noise there is harmless.
  - LayerNorm rstd = exp(-0.5*ln(var+eps)) keeps ACT on the exp/ln
    table set (3 table loads total: exp/ln -> gelu -> exp/ln).
  - Masking: masked K rows are zeroed via the km scalar in the V
    evacuation and excluded from the softmax denominator (65th V
    column = km01); masked Q rows are zeroed by the final qmask scale.
"""

import numpy as np
import ml_dtypes
from contextlib import ExitStack

import concourse.bass as bass
import concourse.bacc as bacc
import concourse.mybir as mybir
import concourse.tile as tile
from concourse.bass_utils import run_bass_kernel_spmd
from concourse.masks import make_identity

FP = mybir.dt.float32
BF = mybir.dt.bfloat16
F8 = mybir.dt.float8e4
AF = mybir.ActivationFunctionType
ALU = mybir.AluOpType
DRM = mybir.MatmulPerfMode.DoubleRow

DIM = 1024
H = 16
DH = 64
B = 4
N = 1024          # keys per batch
NQ = 512          # queries per core
P = 128
NDT = DIM // P    # 8 feature tiles
NKT = N // P      # 8 key tiles
NQT = NQ // P     # 4 query tiles
NG = 4            # DoubleRow contraction groups (4 x 2 x 128 = 1024)
EPS = 1e-5
ESCALE = 1.0 / 8192.0   # 1/(sqrt(DIM) * 16 * 16)

_CACHED_NC = None


def build_nc():
    nc = bacc.Bacc(None, target_bir_lowering=False, debug=True)
    qt8 = nc.declare_dram_parameter("qt8", [P, NG, 2, NQ], F8, isOutput=False)
    wq8 = nc.declare_dram_parameter("wq8", [P, NG, 2, DIM], F8, isOutput=False)
    kt8 = nc.declare_dram_parameter("kt8", [P, NG, 2, N], F8, isOutput=False)
    wk8 = nc.declare_dram_parameter("wk8", [P, NG, 2, DIM], F8, isOutput=False)
    wv8 = nc.declare_dram_parameter("wv8", [P, NG, 2, DIM], F8, isOutput=False)
    vt8 = nc.declare_dram_parameter("vt8", [P, NG, 2, N], F8, isOutput=False)
    qtb = nc.declare_dram_parameter("qtb", [P, NDT, NQ], BF, isOutput=False)
    wqb = nc.declare_dram_parameter("wqb", [P, NDT, DIM], BF, isOutput=False)
    wo = nc.declare_dram_parameter("wo", [P, NDT, DIM], BF, isOutput=False)
    # maskd: cols 0..7 = km01 key-tile columns, 8..11 = qm01 query-tile cols
    maskd = nc.declare_dram_parameter("maskd", [P, NKT + NQT], FP, isOutput=False)
    out = nc.declare_dram_parameter("out", [NQ, DIM], BF, isOutput=True)

    with ExitStack() as ctx:
        tc = ctx.enter_context(tile.TileContext(nc))
        persist = ctx.enter_context(tc.tile_pool(name="persist", bufs=1))

        # ---- persistent SBUF tiles ----
        mask_sb = persist.tile([P, NKT + NQT], FP, tag="mask", name="mask_sb")
        qt8_sb = persist.tile([P, NG, 2, NQ], F8, tag="qt8", name="qt8_sb")
        wq8_sb = persist.tile([P, NG, 2, DIM], F8, tag="wq8", name="wq8_sb")
        kt8_sb = persist.tile([P, NG, 2, N], F8, tag="kt8", name="kt8_sb")
        wk8_sb = persist.tile([P, NG, 2, DIM], F8, tag="wk8", name="wk8_sb")
        wv8_sb = persist.tile([P, NG, 2, DIM], F8, tag="wv8", name="wv8_sb")
        vt8_sb = persist.tile([P, NG, 2, N], F8, tag="vt8", name="vt8_sb")
        qtb_sb = persist.tile([P, NDT, NQ], BF, tag="qtb", name="qtb_sb")
        wqb_sb = persist.tile([P, NDT, DIM], BF, tag="wqb", name="wqb_sb")
        wo_sb = persist.tile([P, NDT, DIM], BF, tag="wo", name="wo_sb")
        Q8pT = [persist.tile([P, NQ], F8, tag=f"q8pt{i}", name=f"q8pt{i}")
                for i in range(NDT)]
        KpT8 = [persist.tile([P, N], F8, tag=f"kpt8{i}", name=f"kpt8{i}")
                for i in range(NDT)]
        QDR = [persist.tile([64, 2, NQ], F8, tag=f"qdr{i}", name=f"qdr{i}")
               for i in range(NDT)]
        KDR = [persist.tile([64, 2, N], F8, tag=f"kdr{i}", name=f"kdr{i}")
               for i in range(NDT)]
        # VDR[jp]: slot t = key-tile 2jp+t; per head 66 cols (64 dims,
        # col 64 = km01 denominator column, col 65 pad)
        VDR = [persist.tile([P, 2, H, DH + 2], F8, tag=f"vdr{j}", name=f"vdr{j}")
               for j in range(NKT // 2)]
        Qp = [persist.tile([P, DIM], BF, tag=f"qp{t}", name=f"qp{t}")
              for t in range(NQT)]
        Ob = persist.tile([P, NQT, DIM], BF, tag="ob", name="ob")
        O1 = [persist.tile([P, DIM], BF, tag=f"o1_{t}", name=f"o1_{t}")
              for t in range(NQT)]
        OTb = persist.tile([P, NDT, NQ], BF, tag="otb", name="otb")
        identb = persist.tile([P, P], BF, tag="identb", name="identb")
        make_identity(nc, identb)
        eps_sb = persist.tile([P, 1], FP, tag="eps", name="eps_sb")
        nc.vector.memset(eps_sb, EPS)

        km = mask_sb[:, 0:NKT]
        qm = mask_sb[:, NKT:NKT + NQT]

        # ---- input DMAs (SP, issue order = priority order) ----
        nc.sync.dma_start(out=mask_sb, in_=maskd[:, :])
        nc.sync.dma_start(out=qt8_sb, in_=qt8[:, :, :, :])
        nc.sync.dma_start(out=wq8_sb, in_=wq8[:, :, :, :])
        nc.sync.dma_start(out=kt8_sb, in_=kt8[:, :, :, :])
        nc.sync.dma_start(out=wk8_sb[:, :, :, 0:512], in_=wk8[:, :, :, 0:512])
        nc.sync.dma_start(out=wk8_sb[:, :, :, 512:1024], in_=wk8[:, :, :, 512:1024])
        nc.sync.dma_start(out=wv8_sb, in_=wv8[:, :, :, :])
        nc.sync.dma_start(out=vt8_sb, in_=vt8[:, :, :, :])
        # qtb/wqb/wo are issued AFTER the QDR/KDR shuffles below: they are
        # not needed until ~25us in, and the in-flight DMA queue-capacity
        # waits would otherwise stall the latency-critical shuffles.

        # ---- phase 1a: scores-Q projection (fp8 DR) ----
        with tc.tile_pool(name="q8ps", bufs=2, space="PSUM") as q8ps:
            for i in range(NDT):
                ps = q8ps.tile([P, NQ], FP, tag="ps", name=f"q8ps{i}")
                for g in range(NG):
                    nc.tensor.matmul(ps, wq8_sb[:, g, :, P * i:P * (i + 1)],
                                     qt8_sb[:, g, :, :],
                                     start=(g == 0), stop=(g == NG - 1),
                                     perf_mode=DRM)
                nc.vector.tensor_copy(Q8pT[i], ps)

        # ---- phase 1b: K projection (fp8 DR) ----
        with tc.tile_pool(name="kps", bufs=2, space="PSUM") as kps:
            for i in range(NDT):
                ps = kps.tile([P, 2, 512], FP, tag="ps", name=f"kps{i}")
                for c in range(2):
                    for g in range(NG):
                        nc.tensor.matmul(ps[:, c, :],
                                         wk8_sb[:, g, :, P * i:P * (i + 1)],
                                         kt8_sb[:, g, :, 512 * c:512 * c + 512],
                                         start=(g == 0), stop=(g == NG - 1),
                                         perf_mode=DRM)
                nc.vector.tensor_copy(KpT8[i], ps.rearrange("p c n -> p (c n)"))

        # ---- DR-layout shuffles (partition p -> (p//2, slot p%2)) ----
        for i in range(NDT):
            nc.sync.dma_start(out=QDR[i], in_=Q8pT[i][:, :])
        for i in range(NDT):
            nc.sync.dma_start(out=KDR[i], in_=KpT8[i][:, :])
        nc.sync.dma_start(out=qtb_sb, in_=qtb[:, :, :])
        nc.sync.dma_start(out=wqb_sb[:, 0:4], in_=wqb[:, 0:4, :])
        nc.sync.dma_start(out=wqb_sb[:, 4:8], in_=wqb[:, 4:8, :])
        nc.sync.dma_start(out=wo_sb[:, 0:4], in_=wo[:, 0:4, :])
        nc.sync.dma_start(out=wo_sb[:, 4:8], in_=wo[:, 4:8, :])

        # ---- phase 2: attention (+ V proj and Qp proj interleaved) ----
        es_tiles = {}
        LAG = 2

        with tc.tile_pool(name="spp", bufs=2, space="PSUM") as spp, \
             tc.tile_pool(name="avp", bufs=1, space="PSUM") as avp, \
             tc.tile_pool(name="tpp", bufs=1, space="PSUM") as tpp, \
             tc.tile_pool(name="esp", bufs=3) as esp, \
             tc.tile_pool(name="p2sb", bufs=2) as p2sb:

            vctx = ExitStack()
            vps = vctx.enter_context(tc.tile_pool(name="vps", bufs=1, space="PSUM"))
            qpctx = ExitStack()
            qpps = None

            def scores_head(h):
                i, g = h // 2, 32 * (h % 2)
                for jp in range(NKT // 2):
                    sp = spp.tile([P, 2, NQ], FP, tag="sp", name=f"sp{h}_{jp}")
                    for s in range(2):
                        k = 2 * jp + s
                        nc.tensor.matmul(sp[:, s, :],
                                         KDR[i][g:g + 32, :, P * k:P * (k + 1)],
                                         QDR[i][g:g + 32, :, :],
                                         start=True, stop=True, perf_mode=DRM)
                    es = esp.tile([P, 2, NQ], F8, tag=f"es{jp}", name=f"es{h}_{jp}")
                    nc.scalar.activation(out=es, in_=sp, func=AF.Exp, scale=ESCALE)
                    es_tiles[(h, jp)] = es

            def av_head(h):
                av = avp.tile([DH + 1, NQ], FP, tag="av", name=f"av{h}")
                for jp in range(NKT // 2):
                    nc.tensor.matmul(av, VDR[jp][:, :, h, 0:DH + 1],
                                     es_tiles.pop((h, jp)),
                                     start=(jp == 0), stop=(jp == NKT // 2 - 1),
                                     perf_mode=DRM)
                avsb = p2sb.tile([DH + 1, NQ], BF, tag="avsb", name=f"avsb{h}")
                nc.vector.tensor_copy(avsb, av)
                tpg = tpp.tile([P, NQT, DH + 2], BF, tag="tpg", name=f"tpg{h}")
                for t in range(NQT):
                    nc.tensor.matmul(tpg[:, t, 0:DH + 1],
                                     avsb[:, P * t:P * (t + 1)],
                                     identb[0:DH + 1, 0:DH + 1],
                                     is_transpose=True,
                                     start=(t == 0), stop=(t == NQT - 1))
                osb = p2sb.tile([P, NQT, DH + 2], BF, tag="osb", name=f"osb{h}")
                nc.vector.tensor_copy(osb[:, :, 0:DH + 1], tpg[:, :, 0:DH + 1])
                dr = p2sb.tile([P, NQT, 1], BF, tag="dr", name=f"dr{h}")
                with nc.allow_low_precision(
                        reason="denom ~512, bf16 recip err 0.4% on a term 30x "
                               "smaller than the residual"):
                    nc.vector.reciprocal(out=dr, in_=osb[:, :, DH:DH + 1])
                nc.vector.tensor_mul(Ob[:, :, DH * h:DH * (h + 1)],
                                     osb[:, :, 0:DH],
                                     dr.to_broadcast((P, NQT, DH)))

            def v_proj():
                for i in range(NKT):
                    for c in range(2):
                        ps = vps.tile([P, 512], FP, tag="ps", name=f"vps{i}_{c}")
                        for g in range(NG):
                            nc.tensor.matmul(ps, vt8_sb[:, g, :, P * i:P * (i + 1)],
                                             wv8_sb[:, g, :, 512 * c:512 * (c + 1)],
                                             start=(g == 0), stop=(g == NG - 1),
                                             perf_mode=DRM)
                        nc.vector.tensor_scalar_mul(
                            out=VDR[i // 2][:, i % 2, 8 * c:8 * c + 8, 0:DH],
                            in0=ps.rearrange("p (h d) -> p h d", h=8),
                            scalar1=km[:, i:i + 1])
                # denominator columns (km01, excluded keys contribute 0)
                for jp in range(NKT // 2):
                    nc.vector.tensor_copy(
                        VDR[jp][:, :, :, DH:DH + 1],
                        km[:, 2 * jp:2 * jp + 2].to_broadcast((P, 2, H, 1)))

            def qp_proj_part(t):
                for c in range(2):
                    ps = qpps.tile([P, 512], FP, tag="ps", name=f"qpp{t}_{c}")
                    for j in range(NDT):
                        nc.tensor.matmul(ps, qtb_sb[:, j, P * t:P * (t + 1)],
                                         wqb_sb[:, j, 512 * c:512 * (c + 1)],
                                         start=(j == 0), stop=(j == NDT - 1))
                    nc.vector.tensor_copy(Qp[t][:, 512 * c:512 * (c + 1)], ps)

            for h in range(H):
                scores_head(h)
                if h == 1:
                    v_proj()
                if h == 5:
                    vctx.close()
                    qpps = qpctx.enter_context(
                        tc.tile_pool(name="qpps", bufs=2, space="PSUM"))
                if h >= LAG:
                    av_head(h - LAG)
                if 6 <= h <= 9:
                    qp_proj_part(h - 6)
            for h in range(H - LAG, H):
                av_head(h)
            qpctx.close()

        # ---- phase 3: residual + LN1 + fc_o + GELU + LN2 ----
        with tc.tile_pool(name="p3", bufs=1) as p3, \
             tc.tile_pool(name="p3s", bufs=2) as p3s, \
             tc.tile_pool(name="tpp3", bufs=2, space="PSUM") as tpp3, \
             tc.tile_pool(name="fps", bufs=2, space="PSUM") as fps:

            def ln_stats(x_ap, mv_ap, tag):
                stats = p3s.tile([P, 2, 6], FP, tag="st", name=f"st{tag}", bufs=4)
                xg = x_ap.rearrange("p (s d) -> p s d", s=2)
                for s in range(2):
                    nc.vector.bn_stats(out=stats[:, s, :], in_=xg[:, s, :])
                nc.vector.bn_aggr(out=mv_ap, in_=stats)

            def ln_rstd4(mv_all, tag):
                """Batched over all 4 t: one Sqrt instruction (table-load
                friendly: its deps force it after the last t's stats)."""
                sd4 = p3s.tile([P, NQT, 1], FP, tag="sd4", name=f"sd4{tag}")
                nc.scalar.activation(out=sd4, in_=mv_all[:, :, 1:2], func=AF.Sqrt,
                                     bias=eps_sb[:, 0:1])
                rstd4 = p3s.tile([P, NQT, 1], FP, tag="rstd4", name=f"rstd4{tag}")
                nc.vector.reciprocal(out=rstd4, in_=sd4)
                return rstd4

            mv1 = p3s.tile([P, NQT, 2], FP, tag="mv1", name="mv1")
            r1s = []
            for t in range(NQT):
                r1 = p3s.tile([P, DIM], BF, tag="r1", name=f"r1_{t}", bufs=4)
                nc.vector.tensor_add(r1, Qp[t], Ob[:, t])
                r1s.append(r1)
                ln_stats(r1, mv1[:, t, :], f"a{t}")
            rstd1 = ln_rstd4(mv1, "a")
            for t in range(NQT):
                nc.vector.tensor_scalar(
                    out=O1[t], in0=r1s[t], scalar1=mv1[:, t, 0:1],
                    scalar2=rstd1[:, t], op0=ALU.subtract, op1=ALU.mult)
                tp = tpp3.tile([P, NDT, P], BF, tag="tp3", name=f"tp3_{t}")
                for i in range(NDT):
                    nc.tensor.matmul(tp[:, i, :], O1[t][:, P * i:P * (i + 1)],
                                     identb, is_transpose=True,
                                     start=(i == 0), stop=(i == NDT - 1))
                nc.vector.tensor_copy(OTb[:, :, P * t:P * (t + 1)], tp)

            r2s = []
            for t in range(NQT):
                r2 = p3s.tile([P, DIM], BF, tag="r2", name=f"r2_{t}", bufs=4)
                for c in range(2):
                    ps = fps.tile([P, 512], FP, tag="fps", name=f"fps{t}_{c}")
                    for i in range(NDT):
                        nc.tensor.matmul(ps, OTb[:, i, P * t:P * (t + 1)],
                                         wo_sb[:, i, 512 * c:512 * (c + 1)],
                                         start=(i == 0), stop=(i == NDT - 1))
                    g = p3s.tile([P, 512], BF, tag="g", name=f"g{t}_{c}", bufs=4)
                    nc.scalar.activation(out=g, in_=ps, func=AF.Gelu)
                    nc.vector.tensor_add(r2[:, 512 * c:512 * (c + 1)],
                                         O1[t][:, 512 * c:512 * (c + 1)], g)
                r2s.append(r2)

            mv2 = p3s.tile([P, NQT, 2], FP, tag="mv2", name="mv2")
            for t in range(NQT):
                ln_stats(r2s[t], mv2[:, t, :], f"b{t}")
            rstd2 = ln_rstd4(mv2, "b")
            nc.vector.tensor_mul(rstd2, rstd2,
                                 qm.rearrange("p (t o) -> p t o", o=1))
            for t in range(NQT):
                fin = p3s.tile([P, DIM], BF, tag="fin", name=f"fin_{t}", bufs=4)
                nc.vector.tensor_scalar(
                    out=fin, in0=r2s[t], scalar1=mv2[:, t, 0:1],
                    scalar2=rstd2[:, t], op0=ALU.subtract, op1=ALU.mult)
                nc.sync.dma_start(out=out[P * t:P * (t + 1), :], in_=fin)

    nc.compile()
    return nc


def _get_nc():
    global _CACHED_NC
    if _CACHED_NC is None:
        _CACHED_NC = build_nc()
    return _CACHED_NC


E4NP = ml_dtypes.float8_e4m3
BFNP = ml_dtypes.bfloat16


def _g_pack(m):
    """[1024 (din), cols] -> [128, 4, 2, cols] DoubleRow group layout."""
    cols = m.shape[1]
    return np.ascontiguousarray(
        m.reshape(NG, 2, P, cols).transpose(2, 0, 1, 3))


def _j_pack(m):
    """[8*128 rows, cols] -> [128, 8, cols] row-tile layout."""
    cols = m.shape[1]
    return np.ascontiguousarray(
        m.reshape(NDT, P, cols).transpose(1, 0, 2))


def _make_in_maps(inputs):
    Q, K, V = inputs["Q"], inputs["K"], inputs["V"]
    mask_Q, mask_K = inputs["mask_Q"], inputs["mask_K"]
    wqT = np.ascontiguousarray(inputs["Wq"].T.astype(np.float32))
    wkT = np.ascontiguousarray(inputs["Wk"].T.astype(np.float32))
    wvT = np.ascontiguousarray(inputs["Wv"].T.astype(np.float32))
    woT = np.ascontiguousarray(inputs["Wo"].T.astype(np.float32))

    wq8 = _g_pack(wqT * 16.0).astype(E4NP)
    wk8 = _g_pack(wkT * 16.0).astype(E4NP)
    wv8 = _g_pack(wvT).astype(E4NP)
    wqb = _j_pack(wqT).astype(BFNP)
    wob = _j_pack(woT).astype(BFNP)

    in_maps = []
    for c in range(8):
        b, q0 = c // 2, (c % 2) * NQ
        kT = np.ascontiguousarray(K[b].T)
        vT = np.ascontiguousarray(V[b].T)
        qT = np.ascontiguousarray(Q[b, q0:q0 + NQ, :].T)
        km01 = np.where(mask_K[b], 0.0, 1.0).astype(np.float32)
        qm01 = np.where(mask_Q[b, q0:q0 + NQ], 0.0, 1.0).astype(np.float32)
        maskd = np.concatenate([km01.reshape(NKT, P).T,
                                qm01.reshape(NQT, P).T], axis=1)
        in_maps.append({
            "qt8": _g_pack(qT).astype(E4NP),
            "wq8": wq8,
            "kt8": _g_pack(kT).astype(E4NP),
            "wk8": wk8,
            "wv8": wv8,
            "vt8": _g_pack(vT).astype(E4NP),
            "qtb": _j_pack(qT).astype(BFNP),
            "wqb": wqb,
            "wo": wob,
            "maskd": np.ascontiguousarray(maskd),
        })
    return in_maps


def _assemble(results):
    out = np.empty((B, 1024, DIM), np.float32)
    for c in range(8):
        b, q0 = c // 2, (c % 2) * NQ
        out[b, q0:q0 + NQ, :] = results[c]["out"].astype(np.float32)
    return out


def kernel(**inputs):
    nc = _get_nc()
    res = run_bass_kernel_spmd(nc, _make_in_maps(inputs), core_ids=list(range(8)))
    return _assemble(res.results)


def kernel_profiled(inputs, **kw):
    nc = _get_nc()
    res = run_bass_kernel_spmd(nc, _make_in_maps(inputs),
                               core_ids=list(range(8)), trace=True, **kw)
    return _assemble(res.results), res
